# revision 47
# baseline (speedup 1.0000x reference)
"""Two-layer GCN encoder on 8 Trainium2 NeuronCores.

Strategy (dst-partitioned, matmul-based segment sum, fp16 internal):
  - Nodes are grouped into 392 blocks of 128; blocks are assigned to
    (core, slot) pairs balancing edge counts, 49 slots per core.
  - Every edge is owned by the core owning its dst block, so each core's
    aggregation for its blocks is complete: no all-reduce needed.
  - Node features live in a chunk-major "AllGather layout": slots are
    grouped into 5 chunks; chunk c holds rows [base_c + core*rows_c +
    (slot-slot0_c)*128 + off].  Both layers gather from this layout with
    the SAME edge indices: layer 1 from a host-precast fp16 copy of x
    (an input parameter - x is replicated, so no on-device cast or
    AllGather), layer 2 from h_full, assembled by 5 chunked AllGathers
    that overlap the tail of layer 1.
  - Per edge tile (128 edges): dma_gather source rows into SBUF
    partitions and accumulate aggT[feat, node] += Xg.T @ P in PSUM.
    The P selection matrices (P[e, n] = (n == dstcol_e) * w_e) are
    precomputed on the host, staged in DRAM, and DMA'd per group - no
    vector-engine work in the main loop, so nothing contends for the
    SBUF port pair that GpSimd needs for SWDGE descriptor generation.
  - Per block: h = relu(aggT.T @ W + b) via two matmuls (bias as a K=1
    matmul) and an ACT relu eviction (fp16 for layer 1, fp32 for the
    final output).
  - dma_gather descriptor generation is spread round-robin across all 4
    SWDGE queues, i.e. all four Q7 core pairs generate concurrently.

dma_gather uses int16 indices (and hangs above ~1024 indices/call), so
gather sources are split at AG row 32768 (lo/hi) and calls are limited
to 8 tiles.
"""

import numpy as np
from concourse import bacc, bass, mybir, tile
from concourse.bass_utils import run_bass_kernel_spmd

P = 128
N_NODES = 50000
N_EDGES = 800000
NFEAT = 128
NC = 8
SLOTS = 49                 # node blocks per core
NB = NC * SLOTS            # 392 blocks, 50176 padded rows
SHARD = SLOTS * P          # 6272 rows per core
NFULL = NB * P             # 50176
GROUP = 3                  # slots per gather group
CALL_TILES = 8             # dma_gather hangs above ~1024 idxs/call

FP32 = mybir.dt.float32
FP16 = mybir.dt.float16

# AllGather chunking: two chunks; the gather lo/hi source split coincides
# with the chunk boundary and each chunk is one Shared-output AllGather.
# 24/25 split (instead of the int16-limit 32/17): the lo AllGather's inputs
# complete at ~half of layer 1, so it lands as layer-1 gathers end, and
# layer-2's lo gathers (~half the work) then cover the hi AllGather.
# Both chunks stay under the 32768-row int16 dma_gather index limit.
LO_SLOTS = 24
LO_SPLIT = NC * LO_SLOTS * P        # 24576 = chunk-0 rows
CHUNKS = [
    {"s0": 0, "ns": LO_SLOTS, "rows": LO_SLOTS * P, "base": 0},
    {"s0": LO_SLOTS, "ns": SLOTS - LO_SLOTS, "rows": (SLOTS - LO_SLOTS) * P,
     "base": LO_SPLIT},
]
assert CHUNKS[1]["base"] + NC * CHUNKS[1]["rows"] == NFULL

# Set by kernel() for test harness introspection (trace results etc.)
last_run_results = None


def _ag_row(core, slot, off):
    """Chunk-major AllGather-layout row for (core, slot, off). Vectorized."""
    slot = np.asarray(slot)
    ci = (slot >= LO_SLOTS).astype(np.int64)
    base = np.array([c["base"] for c in CHUNKS])[ci]
    rows = np.array([c["rows"] for c in CHUNKS])[ci]
    s0 = np.array([c["s0"] for c in CHUNKS])[ci]
    return base + core * rows + (slot - s0) * P + off


def _wrap16(flat):
    """dma_gather index layout: logical i -> [i % 16, i // 16], x8 replicated."""
    n16 = len(flat) // 16
    arr = np.asarray(flat, dtype=np.int16).reshape(n16, 16).T  # [16, n16]
    return np.tile(arr, (8, 1))  # [128, n16]


def _prep(edge_index, edge_weight):
    """Host-side sharding: block assignment, gather indices, pm tiles."""
    src = edge_index[0].astype(np.int64)
    dst = edge_index[1].astype(np.int64)
    w = edge_weight.astype(np.float32)

    blk = dst >> 7
    col = (dst & 127).astype(np.int64)

    cnt = np.bincount(blk, minlength=NB)
    order = np.argsort(-cnt, kind="stable")
    # Refine within slabs of 4 slots: re-sort by lo-edge count so each
    # slot's 8 blocks have similar lo/hi splits (reduces the shared
    # max-over-cores tile schedule).  The AG-row threshold depends on the
    # assignment itself, so approximate lo-ness with a first-pass
    # assignment by total count.
    core_of0 = np.empty(NB, np.int64)
    slot_of0 = np.empty(NB, np.int64)
    ba0 = order.reshape(SLOTS, NC).T
    for c0 in range(NC):
        for s0 in range(SLOTS):
            core_of0[ba0[c0, s0]] = c0
            slot_of0[ba0[c0, s0]] = s0
    sblk0 = src >> 7
    v0 = _ag_row(core_of0[sblk0], slot_of0[sblk0], src & 127)
    lo_cnt = np.bincount(blk[v0 < LO_SPLIT], minlength=NB)
    order2 = order.copy()
    for a in range(0, NB, 4 * NC):
        slab = order2[a:a + 4 * NC]
        order2[a:a + 4 * NC] = slab[np.argsort(-lo_cnt[slab], kind="stable")]
    block_at = order2.reshape(SLOTS, NC).T          # [core, slot] -> block
    core_of = np.empty(NB, np.int64)
    slot_of = np.empty(NB, np.int64)
    for c in range(NC):
        for s in range(SLOTS):
            core_of[block_at[c, s]] = c
            slot_of[block_at[c, s]] = s

    eorder = np.argsort(blk, kind="stable")
    estart = np.zeros(NB + 1, np.int64)
    np.cumsum(cnt, out=estart[1:])

    # gather index (AllGather-layout row) for each edge's src
    sblk = src >> 7
    v = _ag_row(core_of[sblk], slot_of[sblk], src & 127)

    groups = [list(range(g, min(g + GROUP, SLOTS))) for g in range(0, SLOTS, GROUP)]

    # per (core, slot): lo/hi edge id lists + shared tile schedule
    ids_cs = [[None] * SLOTS for _ in range(NC)]
    LT = np.zeros(SLOTS, np.int64)
    HT = np.zeros(SLOTS, np.int64)
    for c in range(NC):
        for s in range(SLOTS):
            b = block_at[c, s]
            ids = eorder[estart[b]:estart[b + 1]]
            m = v[ids] < LO_SPLIT
            lo, hi = ids[m], ids[~m]
            ids_cs[c][s] = (lo, hi)
            LT[s] = max(LT[s], (len(lo) + P - 1) // P)
            HT[s] = max(HT[s], (len(hi) + P - 1) // P)

    # Tile enumeration: for g in groups: for part in (lo, hi): for s in g.
    gdescs = []
    tid0 = 0
    lo_off = 0
    for g in groups:
        lo_tiles = int(sum(LT[s] for s in g))
        hi_tiles = int(sum(HT[s] for s in g))
        gdescs.append({
            "slots": g, "lo_tiles": lo_tiles, "hi_tiles": hi_tiles, "tid0": tid0,
            "lo_off": lo_off,
        })
        tid0 += lo_tiles + hi_tiles
        lo_off += lo_tiles
    sched = {"LT": LT, "HT": HT, "groups": gdescs, "ntiles": tid0,
             "lo_total": lo_off}

    # Gather calls: one per (group, part, <=CALL_TILES window).  Idx streams
    # are padded with -1 (the Q7 ucode trims trailing negatives, so padding
    # costs no descriptors).
    calls = []  # (group, part, tile0_in_group_part, ntiles)
    for gi, g in enumerate(groups):
        for part in range(2):
            T = LT if part == 0 else HT
            ptiles = int(sum(T[s] for s in g))
            t0 = 0
            while t0 < ptiles:
                nt = min(CALL_TILES, ptiles - t0)
                calls.append((gi, part, t0, nt))
                t0 += nt
    sched["calls"] = calls

    idx_np = []
    pm_np = []
    colw_np = []
    for c in range(NC):
        flat_idx = []
        # lo tiles: pm dense, partition-major [row-in-tile, lo_tile*128 +
        # dstcol], packed per group so each group's lo tiles load as one
        # contiguous-per-partition HWDGE DMA.  hi tiles: compact (col, w)
        # scalars, expanded on-device by the vector engine (halves the pm
        # HBM traffic, which otherwise saturates HBM alongside the gathers).
        pm = np.zeros((P, lo_off * P), np.float16)
        colw = np.zeros((P, 2 * tid0), np.float32)
        tid = 0
        lo_tid = 0
        for g in groups:
            for part in range(2):
                T = LT if part == 0 else HT
                for s in g:
                    lo, hi = ids_cs[c][s]
                    ids = lo if part == 0 else hi
                    n = int(T[s]) * P
                    ne = len(ids)
                    iv = np.zeros(n, np.int64)
                    iv[:ne] = v[ids] - (0 if part == 0 else LO_SPLIT)
                    flat_idx.append(iv)
                    e = np.arange(ne)
                    if part == 0:
                        pm[e % P, (lo_tid + e // P) * P + col[ids]] = w[ids]
                        lo_tid += int(T[s])
                    else:
                        cv = np.zeros(n, np.float32)
                        wv = np.zeros(n, np.float32)
                        cv[:ne] = col[ids]
                        wv[:ne] = w[ids]
                        for t in range(int(T[s])):
                            colw[:, 2 * (tid + t)] = cv[t * P:(t + 1) * P]
                            colw[:, 2 * (tid + t) + 1] = wv[t * P:(t + 1) * P]
                    tid += int(T[s])
        idx_np.append(_wrap16(np.concatenate(flat_idx)))
        pm_np.append(pm)
        colw_np.append(colw)

    return block_at, sched, idx_np, pm_np, colw_np


def _build(sched, n16):
    """Build the SPMD bass program. Returns finalized nc."""
    nc = bacc.Bacc(num_devices=NC, num_swdge_queues=4)

    x16_in = nc.declare_dram_parameter("x16", [NFULL, NFEAT], FP16, isOutput=False)
    w1_in = nc.declare_dram_parameter("W1", [NFEAT, NFEAT], FP32, isOutput=False)
    w2_in = nc.declare_dram_parameter("W2", [NFEAT, NFEAT], FP32, isOutput=False)
    b1_in = nc.declare_dram_parameter("b1", [1, NFEAT], FP32, isOutput=False)
    b2_in = nc.declare_dram_parameter("b2", [1, NFEAT], FP32, isOutput=False)
    idx_in = nc.declare_dram_parameter("idx", [P, n16], mybir.dt.int16, isOutput=False)
    pm_in = nc.declare_dram_parameter(
        "pm", [P, sched["lo_total"] * P], FP16, isOutput=False
    )
    colw_in = nc.declare_dram_parameter(
        "colw", [P, 2 * sched["ntiles"]], FP32, isOutput=False
    )
    iota_in = nc.declare_dram_parameter("iota", [P, P], FP32, isOutput=False)
    out = nc.declare_dram_parameter("out", [SHARD, NFEAT], FP32, isOutput=True)

    relu = mybir.ActivationFunctionType.Relu

    with tile.TileContext(nc) as tc:
        with tc.tile_pool(name="const", bufs=1) as cpool, \
             tc.tile_pool(name="gbuf", bufs=7) as gpool, \
             tc.tile_pool(name="pmg", bufs=6) as pmpool, \
             tc.tile_pool(name="pmat", bufs=12) as ppool, \
             tc.tile_pool(name="evict", bufs=3) as epool, \
             tc.tile_pool(name="hout", bufs=3) as hpool, \
             tc.tile_pool(name="psA", bufs=5, space="PSUM") as psA, \
             tc.tile_pool(name="psB", bufs=2, space="PSUM") as psB, \
             tc.tile_pool(name="psC", bufs=1, space="PSUM") as psC, \
             tc.tile_pool(name="dram", bufs=1, space="DRAM") as dpool:

            w_t = [cpool.tile([P, P], FP16, name=f"w{l}") for l in range(2)]
            b_t = [cpool.tile([1, P], FP16, name=f"b{l}") for l in range(2)]
            wld_t = [cpool.tile([P, P], FP32, name=f"wld{l}") for l in range(2)]
            bld_t = [cpool.tile([1, P], FP32, name=f"bld{l}") for l in range(2)]
            ones_t = cpool.tile([1, P], FP16)
            idx_t = cpool.tile([P, n16], mybir.dt.int16)
            colw_t = cpool.tile([P, 2 * sched["ntiles"]], FP32)
            iota_t = cpool.tile([P, P], FP32)

            # iota in PSUM: the hi-tile P-build tensor_scalar then runs in 1x
            # mode off the PSUM read port and never takes the SBUF port pair
            # GpSimd needs for SWDGE descriptor writes.
            iota_ps = psC.tile([P, P], FP32, space="PSUM", name="iotaps")
            nc.sync.dma_start(out=iota_t[:], in_=iota_in[:])
            nc.vector.tensor_copy(out=iota_ps[:], in_=iota_t[:])
            nc.sync.dma_start(out=colw_t[:], in_=colw_in[:])
            for l, (wi, bi) in enumerate([(w1_in, b1_in), (w2_in, b2_in)]):
                nc.sync.dma_start(out=wld_t[l][:], in_=wi[:])
                nc.sync.dma_start(out=bld_t[l][:], in_=bi[:])
                nc.scalar.copy(out=w_t[l][:], in_=wld_t[l][:])
                nc.scalar.copy(out=b_t[l][:], in_=bld_t[l][:])
            nc.vector.memset(ones_t[:], 1.0)
            nc.sync.dma_start(out=idx_t[:], in_=idx_in[:])

            h_shard = dpool.tile([SHARD, NFEAT], FP16, name="h_shard")
            # Two Shared-output AllGather destinations (a Shared tensor admits
            # exactly one writer instruction): lo = slots [0, 32) of every
            # core = gather rows [0, LO_SPLIT), hi = the rest.
            h_lo = dpool.tile(
                [LO_SPLIT, NFEAT], FP16, name="h_lo", addr_space="Shared"
            )
            h_hi = dpool.tile(
                [NFULL - LO_SPLIT, NFEAT], FP16, name="h_hi", addr_space="Shared"
            )

            call_q = [0]  # round-robin SWDGE queue so descriptor generation
                          # spreads across all four Q7 core pairs

            def layer(l, src_lo, src_hi, dst_ap, out_dt, shift=1,
                      after_consume=None, after_lo=None):
                LT, HT = sched["LT"], sched["HT"]
                groups = sched["groups"]

                def emit_calls(gi, gbuf, want_part):
                    gd = groups[gi]
                    lo_tiles = gd["lo_tiles"]
                    for cgi, cpart, ct0, cnt in sched["calls"]:
                        if cgi != gi or cpart != want_part:
                            continue
                        pos = (0 if cpart == 0 else lo_tiles) + ct0
                        gtid = gd["tid0"] + pos
                        nidx = cnt * P
                        srcap = src_lo if cpart == 0 else src_hi
                        nc.gpsimd.dma_gather(
                            out_ap=gbuf[:, pos * P:pos * P + nidx].rearrange(
                                "p (t e) -> p t e", e=P
                            ),
                            in_ap=srcap,
                            idxs_ap=idx_t[:, gtid * 8:gtid * 8 + nidx // 16],
                            num_idxs=nidx,
                            num_idxs_reg=nidx,
                            elem_size=P,
                            queue_num=call_q[0] % 4,
                        )
                        call_q[0] += 1

                def consume(gi, gbuf, pmg):
                    gd = groups[gi]
                    lo_tiles = gd["lo_tiles"]
                    lo_base = 0
                    hi_base = lo_tiles
                    for s in gd["slots"]:
                        nlo, nhi = int(LT[s]), int(HT[s])
                        tlist = [lo_base + t for t in range(nlo)] + \
                                [hi_base + t for t in range(nhi)]
                        lo_base += nlo
                        hi_base += nhi
                        ntot = nlo + nhi
                        aggT = psA.tile([P, P], FP32, space="PSUM", name="aggT", tag="aggT")
                        for k, gt in enumerate(tlist):
                            if gt < lo_tiles:
                                rhs = pmg[:, gt * P:(gt + 1) * P]
                            else:
                                # hi tile: expand compact (col, w) on the DVE
                                tid = gd["tid0"] + gt
                                pmv = ppool.tile([P, P], FP16, name="pmv", tag="pmv")
                                nc.vector.tensor_scalar(
                                    out=pmv[:],
                                    in0=iota_ps[:],
                                    scalar1=colw_t[:, 2 * tid:2 * tid + 1],
                                    scalar2=colw_t[:, 2 * tid + 1:2 * tid + 2],
                                    op0=mybir.AluOpType.is_equal,
                                    op1=mybir.AluOpType.mult,
                                )
                                rhs = pmv[:]
                            nc.tensor.matmul(
                                out=aggT[:],
                                lhsT=gbuf[:, gt * P:(gt + 1) * P],
                                rhs=rhs,
                                start=(k == 0),
                                stop=(k == ntot - 1),
                            )
                        aggT_sb = epool.tile([P, P], FP16, name="evict", tag="evict")
                        nc.scalar.copy(out=aggT_sb[:], in_=aggT[:])
                        h_ps = psB.tile([P, P], FP32, space="PSUM", name="hps", tag="hps")
                        nc.tensor.matmul(
                            out=h_ps[:], lhsT=aggT_sb[:], rhs=w_t[l][:],
                            start=True, stop=False,
                        )
                        nc.tensor.matmul(
                            out=h_ps[:], lhsT=ones_t[0:1, :], rhs=b_t[l][0:1, :],
                            start=False, stop=True,
                        )
                        h_sb = hpool.tile([P, P], out_dt, name="hout", tag=f"hout{l}")
                        nc.scalar.activation(out=h_sb[:], in_=h_ps[:], func=relu)
                        nc.sync.dma_start(
                            out=dst_ap[s * P:(s + 1) * P, :], in_=h_sb[:]
                        )
                    if after_consume is not None:
                        after_consume(gi)

                # Software-pipelined emission: group g's hi calls and compute
                # go out `shift` groups after its lo calls, so (a) consumption
                # runs behind production and (b) in layer 2 the hi calls
                # (waiting on the hi AllGather) never head-of-line-block the
                # next groups' lo calls on the gpsimd stream.
                state = {}  # gi -> (gbuf, pmg)

                def tail(j):
                    pg, pp = state.pop(j)
                    emit_calls(j, pg, 1)
                    consume(j, pg, pp)

                for gi, gd in enumerate(groups):
                    all_tiles = gd["lo_tiles"] + gd["hi_tiles"]
                    gbuf = gpool.tile([P, all_tiles * P], FP16, name="gbuf", tag="gbuf")
                    # lo-tile pm slabs arrive with one HWDGE DMA per group
                    # (contiguous per partition; issued from the SP engine so
                    # descriptor generation never blocks ACT evictions).
                    nlo = gd["lo_tiles"]
                    pmg = pmpool.tile([P, nlo * P], FP16, name="pmg", tag="pmg")
                    nc.sync.dma_start(
                        out=pmg[:, :nlo * P],
                        in_=pm_in[:, gd["lo_off"] * P:(gd["lo_off"] + nlo) * P],
                    )
                    state[gi] = (gbuf, pmg)
                    emit_calls(gi, gbuf, 0)
                    if after_lo is not None:
                        after_lo(gi)
                    if gi - shift >= 0:
                        tail(gi - shift)
                for j in sorted(state):
                    tail(j)

            # Collectives ride the gpsimd stream (the only engine allowed to
            # trigger them), so placement controls head-of-line blocking: an
            # AllGather whose semaphore wait is not yet satisfied stalls every
            # later dma_gather dispatch.  The lo AllGather is emitted one
            # consume-group after its h rows complete (wait ~cleared by then);
            # the hi AllGather is emitted inside layer 2 after two groups of
            # lo calls, which only need the lo AllGather.
            ag_out = [h_lo, h_hi]

            def emit_ag(ci):
                ch = CHUNKS[ci]
                nc.gpsimd.collective_compute(
                    "AllGather", mybir.AluOpType.bypass,
                    replica_groups=[list(range(NC))],
                    ins=[h_shard[ch["s0"] * P:(ch["s0"] + ch["ns"]) * P, :]],
                    outs=[ag_out[ci][:]],
                )

            def l1_after_consume(gi):
                gd = sched["groups"][gi]
                if gd["slots"][-1] + 1 >= CHUNKS[0]["ns"] + GROUP and \
                        not l1_after_consume.done:
                    l1_after_consume.done = True
                    emit_ag(0)

            l1_after_consume.done = False

            def l2_after_lo(gi):
                if gi == 5:
                    emit_ag(1)

            layer(0, x16_in[0:LO_SPLIT, :], x16_in[LO_SPLIT:NFULL, :],
                  h_shard[:], FP16, shift=1, after_consume=l1_after_consume)
            assert l1_after_consume.done

            layer(1, h_lo[:], h_hi[:], out[:], FP32, shift=5,
                  after_lo=l2_after_lo)

    nc.finalize()
    return nc


def kernel(x, edge_index, edge_weight, W1, b1, W2, b2):
    global last_run_results
    x = np.ascontiguousarray(np.asarray(x, dtype=np.float32))
    edge_index = np.asarray(edge_index)
    edge_weight = np.asarray(edge_weight, dtype=np.float32)

    block_at, sched, idx_np, pm_np, colw_np = _prep(edge_index, edge_weight)
    n16 = idx_np[0].shape[1]
    nc = _build(sched, n16)
    iota_np = np.broadcast_to(np.arange(P, dtype=np.float32), (P, P)).copy()

    xpad = np.zeros((NFULL, NFEAT), np.float32)
    xpad[:N_NODES] = x
    # Full x in chunk-major AllGather layout, host-cast to fp16: x is a
    # replicated input, so layer 1's gather source needs no on-device work.
    x16_np = np.zeros((NFULL, NFEAT), np.float16)
    for c in range(NC):
        for ch in CHUNKS:
            s0, ns = ch["s0"], ch["ns"]
            dst0 = ch["base"] + c * ch["rows"]
            blks = block_at[c, s0:s0 + ns]
            x16_np[dst0:dst0 + ns * P] = (
                xpad.reshape(NB, P, NFEAT)[blks].reshape(ns * P, NFEAT)
            )
    in_maps = []
    for c in range(NC):
        in_maps.append({
            "x16": x16_np,
            "W1": np.ascontiguousarray(W1, dtype=np.float32),
            "W2": np.ascontiguousarray(W2, dtype=np.float32),
            "b1": np.ascontiguousarray(b1, dtype=np.float32).reshape(1, NFEAT),
            "b2": np.ascontiguousarray(b2, dtype=np.float32).reshape(1, NFEAT),
            "idx": idx_np[c],
            "pm": pm_np[c],
            "colw": colw_np[c],
            "iota": iota_np,
        })

    import os
    trace = bool(int(os.environ.get("GCN_TRACE", "0")))
    res = run_bass_kernel_spmd(nc, in_maps, list(range(NC)), trace=trace)
    last_run_results = res

    full = np.zeros((NFULL, NFEAT), np.float32)
    for c in range(NC):
        shard = res.results[c]["out"]
        for s in range(SLOTS):
            b = int(block_at[c, s])
            full[b * P:(b + 1) * P] = shard[s * P:(s + 1) * P]
    return full[:N_NODES]
